# revision 25
# baseline (speedup 1.0000x reference)
"""Trainium2 Bass kernel for nn_MixtureExpertsMlp (MoE soft routing).

Contract: kernel(**inputs) takes the FULL unsharded inputs
(x [4,4096,768], phi [4,1024,768], w1 [4,768,3072], b1 [4,3072],
w2 [4,3072,768], b2 [4,768]) and returns the FULL output [4,4096,768].

Sharding (expert+slot parallel over 8 NeuronCores): core c owns expert
e = c // 2 and slot half h = c % 2, i.e. SL = 512 of that expert's 1024
routing slots. Every core sees all tokens. Per core and per batch b:

  L^T[s, n]    = sum_d phi[s, d] x[b, n, d]        (slots on partitions)
  E^T          = exp(L^T)          (softmax max-subtraction skipped: the
                                    logits are ~N(0,1), well within range)
  ddenom[s]    = sum_n E^T[s, n]                    (via ACT accum_out)
  Et[n, s]     = E^T transposed per 128x128 block via PE matmul against a
                 bf16 identity (no ddenom dependency, so the transposes and
                 slot matmuls pipeline INSIDE phase 1, one tile behind exp)
  slotsU^T[d,s]= sum_n x[b, n, d] Et[n, s]          (unnormalized)
  slots^T[d,s] = slotsU^T[d, s] / ddenom[s]  (normalization folded into the
                 PSUM->SBUF copy: tensor_mul against a broadcast 1/ddenom
                 row, built via PE transpose + 1-partition ones matmul)
  h^T[h', s]   = gelu_tanh(sum_d w1[d, h'] slots^T[d, s] + b1[h'])
  y[s, od]     = sum_h h^T[h, s] w2[h, od]   (s on partitions directly:
                                    lhsT = h^T block, rhs = w2 rows)
  outp[n, :D]  = sum_s E^T[s, n] y[s, :]      (unnormalized combine)
  outp[n, D]   = sum_s E^T[s, n]              (ones column appended to y)

Host-side unshard: the combine softmax normalizer is global over all
E*S slots, so out = (sum_c num_c + sum_c gdl_c * b2[e(c)]) / sum_c gdl_c
where num_c = outp_c[..., :D] and gdl_c = outp_c[..., D]. This also
folds in b2 exactly (per-expert combine mass times b2[e]).

All matmul operands are bf16 (PSUM accumulation stays f32): LDWEIGHTS
runs at full rate, 128-free matmuls run at 1 cyc/row (fp32r needs
free >= 256), DMA traffic and SBUF footprint halve. phi/w1/w2 are
SBUF-resident across batches. Emission is software-pipelined
(transpose(v+1) before slots(v), mlp1(h+1) before mlp2(h)) to keep the
PE matmul pipe gap-free (gaps de-ramp the PE clock 2.4 -> 1.2 GHz).
"""

import numpy as np
import ml_dtypes
from contextlib import ExitStack

import concourse.bass as bass
import concourse.tile as tile
from concourse import mybir
from concourse.bass import ts
from concourse.bass_utils import run_bass_kernel_spmd

F32 = mybir.dt.float32
BF16 = mybir.dt.bfloat16
AF = mybir.ActivationFunctionType

N_CORES = 8


# --------------------------------------------------------------------------
# Post-pass: the walrus build in this container enforces the ISA cap of one
# sync-wait per instruction (two for EventSemaphore); Tile's final drain can
# carry more. Hoist excess waits onto fresh same-engine NOPs.
# --------------------------------------------------------------------------
def _split_excess_waits(nc):
    caps = {"InstEventSemaphore": 2}
    n_new = 0
    for f in nc.m.functions:
        for bb in f.blocks:
            i = 0
            insts = bb.instructions
            while i < len(insts):
                ins = insts[i]
                si = ins.sync_info
                cap = caps.get(type(ins).__name__, 1)
                if si is not None and len(si.on_wait) > cap:
                    waits = list(si.on_wait)
                    keep, hoist = waits[-cap:], waits[:-cap]
                    new_nops = []
                    for w in hoist:
                        nop = mybir.InstNoOp(
                            name=nc.get_next_instruction_name(),
                            engine=ins.engine,
                            ins=[],
                            outs=[],
                            sync_info=mybir.SyncInfo(on_wait=[w], on_update=[]),
                        )
                        nc.register_instruction(nop)
                        new_nops.append(nop)
                    ins.sync_info = mybir.SyncInfo(
                        on_wait=keep, on_update=list(si.on_update)
                    )
                    insts[i:i] = new_nops
                    i += len(new_nops)
                    n_new += len(new_nops)
                i += 1
    return n_new


def _bank_splits(off, width, bank=512):
    """Split [off, off+width) at PSUM-bank (512 f32) boundaries."""
    out, cur = [], off
    while cur < off + width:
        nxt = min((cur // bank + 1) * bank, off + width)
        out.append((cur, nxt - cur))
        cur = nxt
    return out


def _emit_moe_kernel(nc, B, N, D, SL, H, act_fn=AF.Gelu_apprx_tanh):
    assert N % 512 == 0 and D % 128 == 0 and SL % 128 == 0 and H % 128 == 0
    Dc, SLc, Hc = D // 128, SL // 128, H // 128
    NT, NV = N // 512, N // 128
    OD = D + 2  # ones column (combine denom) + pad for alignment

    xT = nc.dram_tensor("xT", [B, Dc, 128, N], BF16, kind="ExternalInput").ap()
    xN = nc.dram_tensor("xN", [B, N, D], BF16, kind="ExternalInput").ap()
    phiT = nc.dram_tensor("phiT", [Dc, 128, SL], BF16, kind="ExternalInput").ap()
    w1 = nc.dram_tensor("w1", [D, H], BF16, kind="ExternalInput").ap()
    w2 = nc.dram_tensor("w2", [H, D], BF16, kind="ExternalInput").ap()
    b1 = nc.dram_tensor("b1", [Hc, 128], F32, kind="ExternalInput").ap()
    identI = nc.dram_tensor("identI", [128, 128], F32, kind="ExternalInput").ap()
    identIb = nc.dram_tensor("identIb", [128, 128], BF16, kind="ExternalInput").ap()
    outp = nc.dram_tensor("outp", [B, N, OD], BF16, kind="ExternalOutput").ap()

    with tile.TileContext(nc) as tc, ExitStack() as ctx:
        pool = lambda name, bufs, space="SBUF": ctx.enter_context(
            tc.tile_pool(name=name, bufs=bufs, space=space)
        )
        singles = pool("singles", 1)
        eT_pool = pool("eT", 1)
        xt_pool = pool("xT", 2)
        xn_pool = pool("xN", 8)
        Dt_pool = pool("Dt", 8)
        slots_pool = pool("slots", 1)
        h_pool = pool("h", 3)
        y_pool = pool("y", 1)
        dd_pool = pool("dd", 2)
        norm_pool = pool("norm", 1)
        out_pool = pool("out", 6)

        # PSUM pools are opened per phase (stack allocator + overlap-deps
        # on release): each phase gets the accumulator banks it needs and
        # the combine phase reuses the freed banks for a deep pso rotation.
        ACC = Dc * 512
        assert ACC == SLc * 768

        # ---- SBUF residents (loaded once, reused across all batches).
        # w1/w2 DMAs (9.4MB) are deferred into batch 0's first tile so they
        # don't queue ahead of the x tiles the first matmuls need.
        phiT_s = singles.tile([128, Dc, SL], BF16)
        nc.sync.dma_start(phiT_s[:], phiT.rearrange("k p m -> p k m"))
        w1_s = singles.tile([128, Dc, H], BF16)
        w2_s = singles.tile([128, Hc, D], BF16)
        b1_s = singles.tile([128, Hc], F32)
        nc.sync.dma_start(b1_s[:], b1.rearrange("o p -> p o"))
        ident = singles.tile([128, 128], F32)
        nc.sync.dma_start(ident[:], identI)
        ident_b = singles.tile([128, 128], BF16)
        nc.sync.dma_start(ident_b[:], identIb)
        # Broadcast-matmul weights: the PE rounds the K=1 contraction up to
        # 32 partitions and reads them all, so rows 1-31 must be REAL zeros
        # (garbage there would be accumulated into every output row).
        ones1 = singles.tile([32, 128], BF16)
        nc.vector.memset(ones1[:], 0.0)
        nc.vector.memset(ones1[0:1, :], 1.0)
        zbias = singles.tile([128, 1], F32)
        nc.vector.memset(zbias[:], 0.0)

        for b in range(B):
            # ---- phase 1+2 fused: logits/exp, per-block PE transposes, and
            # ---- unnormalized slots accumulation in one dense PE stream ----
            eT = eT_pool.tile([128, SLc, N], BF16)
            ddp = dd_pool.tile([128, SLc, NT], F32)
            ps12 = ctx12 = tc.tile_pool(name="ps12", bufs=2, space="PSUM")
            ps12 = ctx12.__enter__()
            psacc_ctx = tc.tile_pool(name="psacc", bufs=1, space="PSUM")
            ps_acc = psacc_ctx.__enter__()
            ps_small = ps12
            accS = ps_acc.tile([128, ACC], F32, tag="acc", name="accS")
            dts, xns = {}, {}

            def emit_T(v):
                psDt = ps_small.tile([128, 512], F32, tag="pss", name="psD")
                for s in range(SLc):
                    nc.tensor.matmul(
                        psDt[:, ts(s, 128)],
                        eT[:, s, ts(v, 128)],
                        ident_b[:],
                        start=True,
                        stop=True,
                    )
                Dt = Dt_pool.tile([128, SL], BF16)
                nc.vector.tensor_copy(Dt[:], psDt[:, :SL])
                dts[v] = Dt

            def emit_S(v):
                Dt, xn = dts.pop(v), xns.pop(v)
                for d in range(Dc):
                    nc.tensor.matmul(
                        accS[:, d * 512 : d * 512 + SL],
                        xn[:, ts(d, 128)],
                        Dt[:],
                        start=(v == 0),
                        stop=(v == NV - 1),
                    )

            # 1/ddenom broadcast-row chain, interleaved into the final tile's
            # T/S pairs so the PE never waits on the DVE/DMA latency:
            #   reduce+recip (DVE, after last exp) -> PE transpose
            #   [128,SLc]->[SLc,128] -> bf16 copy -> flatten DMAs ->
            #   1-partition ones matmul -> broadcast row in SBUF
            rdd = dd_pool.tile([128, SLc], F32, tag="rdd", name="rdd")
            pstS = norm_pool.tile([SLc, 128], BF16, tag="pstS", name="pstS")
            rddF = norm_pool.tile([1, SL], BF16, tag="rddF", name="rddF")
            rddB = norm_pool.tile([128, SL], F32, tag="rddB", name="rddB")

            def norm_chain_a():
                pst = ps_small.tile([128, 512], F32, tag="pss", name="psT")
                nc.tensor.transpose(pst[0:SLc, 0:128], rdd[:], ident[:])
                nc.vector.tensor_copy(pstS[:], pst[0:SLc, 0:128])
                # one DMA per source partition: a single rearranged
                # "p k -> (p k)" DMA under-reports its partition extent to
                # the Tile dependency tracker and races the copy above
                for k in range(SLc):
                    nc.sync.dma_start(rddF[0:1, ts(k, 128)], pstS[k : k + 1, :])

            def norm_chain_b():
                psB = ps_small.tile([128, 512], F32, tag="pss", name="psB")
                nc.tensor.matmul(
                    psB[:, :SL], ones1[0:1, :], rddF[:], start=True, stop=True
                )
                nc.vector.tensor_copy(rddB[:], psB[:, :SL])

            s_queue = []

            def emit_TS_for_tile(t, hooks=()):
                for i, v in enumerate(range(4 * t, 4 * t + 4)):
                    emit_T(v)
                    if s_queue:
                        emit_S(s_queue.pop(0))
                    s_queue.append(v)
                    for hook_i, hook in hooks:
                        if hook_i == i:
                            hook()

            for t in range(NT):
                xt = xt_pool.tile([128, Dc, 512], BF16)
                nc.sync.dma_start(
                    xt[:], xT[b, :, :, ts(t, 512)].rearrange("k p n -> p k n")
                )
                for s in range(SLc):
                    ps = ps_small.tile([128, 512], F32, tag="pss", name="psL")
                    for d in range(Dc):
                        nc.tensor.matmul(
                            ps[:],
                            phiT_s[:, d, ts(s, 128)],
                            xt[:, d, :],
                            start=(d == 0),
                            stop=(d == Dc - 1),
                        )
                    nc.scalar.activation(
                        eT[:, s, ts(t, 512)],
                        ps[:],
                        AF.Exp,
                        bias=zbias[:],
                        accum_out=ddp[:, s, t : t + 1],
                    )
                for v in range(4 * t, 4 * t + 4):
                    xn = xn_pool.tile([128, D], BF16)
                    nc.sync.dma_start(xn[:], xN[b, ts(v, 128), :])
                    xns[v] = xn
                if b == 0:
                    # stream the 9.4MB of resident weights in per-tile chunks
                    # so they never queue ahead of the x tiles phase 1 needs
                    hchunk = H // NT
                    nc.sync.dma_start(
                        w1_s[:, :, ts(t, hchunk)],
                        w1[:, ts(t, hchunk)].rearrange("(k p) m -> p k m", p=128),
                    )
                    kchunk = Hc // NT
                    nc.sync.dma_start(
                        w2_s[:, ts(t, kchunk), :],
                        w2[ts(t, kchunk * 128), :].rearrange(
                            "(k p) m -> p k m", p=128
                        ),
                    )
                if t == NT - 1:
                    nc.vector.reduce_sum(
                        rdd[:], ddp[:], axis=mybir.AxisListType.X
                    )
                    nc.vector.reciprocal(rdd[:], rdd[:])
                if t >= 1:
                    emit_TS_for_tile(t - 1)
            emit_TS_for_tile(NT - 1, hooks=((0, norm_chain_a), (2, norm_chain_b)))
            while s_queue:
                emit_S(s_queue.pop(0))

            # slots^T = slotsU^T * (1/ddenom), fused into the PSUM evacuation
            slotsT = slots_pool.tile([128, Dc, SL], BF16)
            for d in range(Dc):
                nc.vector.tensor_mul(
                    slotsT[:, d, :], accS[:, d * 512 : d * 512 + SL], rddB[:]
                )
            psacc_ctx.__exit__(None, None, None)
            ctx12.__exit__(None, None, None)
            ps3_ctx = tc.tile_pool(name="ps3", bufs=2, space="PSUM")
            ps_small = ps3_ctx.__enter__()
            psaccy_ctx = tc.tile_pool(name="psaccy", bufs=1, space="PSUM")
            ps_acc = psaccy_ctx.__enter__()

            # ---- phase 3: expert MLP; y accumulated with s on partitions ----
            accY = ps_acc.tile([128, ACC], F32, tag="acc", name="accY")

            def emit_h1(h):
                psh = ps_small.tile([128, 512], F32, tag="pss", name="psH")
                for d in range(Dc):
                    nc.tensor.matmul(
                        psh[:, :SL],
                        w1_s[:, d, ts(h, 128)],
                        slotsT[:, d, :],
                        start=(d == 0),
                        stop=(d == Dc - 1),
                    )
                ht = h_pool.tile([128, SL], BF16)
                nc.scalar.activation(
                    ht[:], psh[:, :SL], act_fn, bias=b1_s[:, h : h + 1]
                )
                return ht

            def emit_h2(h, ht):
                for s in range(SLc):
                    for off, sz in _bank_splits(s * D, D):
                        # start=True clears the has_written bits of the WHOLE
                        # 2KB PSUM bank: only the bank-leading piece may carry
                        # it. A same-bank follower piece relies on that clear:
                        # its first start=False matmul overwrites (bit clear),
                        # later ones accumulate (bit set).
                        bank_first = off % 512 == 0
                        nc.tensor.matmul(
                            accY[:, off : off + sz],
                            ht[:, ts(s, 128)],
                            w2_s[:, h, off - s * D : off - s * D + sz],
                            start=(h == 0 and bank_first),
                            stop=(h == Hc - 1),
                            skip_group_check=not bank_first,
                        )

            pend_h = None
            for h in range(Hc):
                cur = emit_h1(h)
                if pend_h is not None:
                    emit_h2(h - 1, pend_h)
                pend_h = cur
            emit_h2(Hc - 1, pend_h)

            # ---- y_aug assembly: y columns + ones column for gdl ----
            y_aug = y_pool.tile([128, SLc, OD], BF16)
            nc.vector.memset(y_aug[:, :, D : D + 1], 1.0)
            nc.vector.memset(y_aug[:, :, D + 1 : D + 2], 0.0)
            for s in range(SLc):
                nc.vector.tensor_copy(
                    y_aug[:, s, :D], accY[:, s * D : (s + 1) * D]
                )

            psaccy_ctx.__exit__(None, None, None)
            ps3_ctx.__exit__(None, None, None)
            ps4_ctx = tc.tile_pool(name="ps4", bufs=6, space="PSUM")
            ps_small = ps4_ctx.__enter__()
            # ---- phase 4: combine partials + local denominator ----
            for v in range(NV):
                ot = out_pool.tile([128, OD], BF16)
                for gi, (off, sz) in enumerate(((0, 512), (512, OD - 512))):
                    pso = ps_small.tile([128, 512], F32, tag="pss", name="psO")
                    for s in range(SLc):
                        nc.tensor.matmul(
                            pso[:, :sz],
                            eT[:, s, ts(v, 128)],
                            y_aug[:, s, off : off + sz],
                            start=(s == 0),
                            stop=(s == SLc - 1),
                        )
                    # alternate evacuation engines so neither serializes
                    # the 2-buf PSUM pool rotation
                    if gi == 0:
                        nc.scalar.copy(ot[:, off : off + sz], pso[:, :sz])
                    else:
                        nc.vector.tensor_copy(ot[:, off : off + sz], pso[:, :sz])
                nc.sync.dma_start(outp[b, ts(v, 128), :], ot[:])
            ps4_ctx.__exit__(None, None, None)

    return nc


def _make_core_inputs(x, phi, w1, b1, w2, n_cores=N_CORES):
    B, N, Dd = x.shape
    E, S, _ = phi.shape
    H = w1.shape[2]
    halves = n_cores // E
    SL = S // halves
    Dc, Hc = Dd // 128, H // 128
    bf = ml_dtypes.bfloat16
    ident_f32 = np.eye(128, dtype=np.float32)
    ident_bf16 = np.eye(128, dtype=bf)
    xT_full = np.ascontiguousarray(
        x.transpose(0, 2, 1).astype(bf)
    ).reshape(B, Dc, 128, N)
    x_c = np.ascontiguousarray(x.astype(bf))
    in_maps = []
    for c in range(n_cores):
        e, hh = c // halves, c % halves
        phi_loc = phi[e, hh * SL : (hh + 1) * SL, :]
        phiT = np.ascontiguousarray(phi_loc.T.astype(bf)).reshape(Dc, 128, SL)
        in_maps.append(
            {
                "xT": xT_full,
                "xN": x_c,
                "phiT": phiT,
                "w1": np.ascontiguousarray(w1[e].astype(bf)),
                "w2": np.ascontiguousarray(w2[e].astype(bf)),
                "b1": np.ascontiguousarray(b1[e]).reshape(Hc, 128),
                "identI": ident_f32,
                "identIb": ident_bf16,
            }
        )
    return in_maps


def _combine_core_outputs(outs, b2, n_cores=N_CORES):
    E, D = b2.shape
    halves = n_cores // E
    num = np.zeros(outs[0]["outp"][..., :D].shape, dtype=np.float64)
    den = np.zeros(outs[0]["outp"][..., D].shape, dtype=np.float64)
    for c, r in enumerate(outs):
        e = c // halves
        gdl = r["outp"][..., D].astype(np.float64)
        num += r["outp"][..., :D]
        num += gdl[..., None] * b2[e].astype(np.float64)[None, None, :]
        den += gdl
    return (num / den[..., None]).astype(np.float32)


def kernel(x, phi, w1, b1, w2, b2):
    x = np.asarray(x, dtype=np.float32)
    phi = np.asarray(phi, dtype=np.float32)
    w1 = np.asarray(w1, dtype=np.float32)
    b1 = np.asarray(b1, dtype=np.float32)
    w2 = np.asarray(w2, dtype=np.float32)
    b2 = np.asarray(b2, dtype=np.float32)

    B, N, D = x.shape
    E, S, _ = phi.shape
    H = w1.shape[2]
    SL = S // (N_CORES // E)

    nc = bass.Bass(
        "TRN2", target_bir_lowering=False, debug=False, num_devices=N_CORES
    )
    _emit_moe_kernel(nc, B, N, D, SL, H)
    _split_excess_waits(nc)

    in_maps = _make_core_inputs(x, phi, w1, b1, w2)
    res = run_bass_kernel_spmd(nc, in_maps, core_ids=list(range(N_CORES)))
    return _combine_core_outputs(res.results, b2)


# revision 26
# speedup vs baseline: 1.1839x; 1.1839x over previous
"""Trainium2 Bass kernel for nn_MixtureExpertsMlp (MoE soft routing).

Contract: kernel(**inputs) takes the FULL unsharded inputs
(x [4,4096,768], phi [4,1024,768], w1 [4,768,3072], b1 [4,3072],
w2 [4,3072,768], b2 [4,768]) and returns the FULL output [4,4096,768].

Sharding (expert+slot parallel over 8 NeuronCores): core c owns expert
e = c // 2 and slot half h = c % 2, i.e. SL = 512 of that expert's 1024
routing slots. Every core sees all tokens. Per core and per batch b:

  L^T[s, n]    = sum_d phi[s, d] x[b, n, d]        (slots on partitions)
  E^T          = exp(L^T)          (softmax max-subtraction skipped: the
                                    logits are ~N(0,1), well within range)
  ddenom[s]    = sum_n E^T[s, n]                    (via ACT accum_out)
  Et[n, s]     = E^T transposed per 128x128 block via PE matmul against a
                 bf16 identity (no ddenom dependency, so the transposes and
                 slot matmuls pipeline INSIDE phase 1, one tile behind exp)
  slotsU^T[d,s]= sum_n x[b, n, d] Et[n, s]          (unnormalized)
  slots^T[d,s] = slotsU^T[d, s] / ddenom[s]  (normalization folded into the
                 PSUM->SBUF copy: tensor_mul against a broadcast 1/ddenom
                 row, built via PE transpose + 1-partition ones matmul)
  h^T[h', s]   = gelu_tanh(sum_d w1[d, h'] slots^T[d, s] + b1[h'])
  y[s, od]     = sum_h h^T[h, s] w2[h, od]   (s on partitions directly:
                                    lhsT = h^T block, rhs = w2 rows)
  outp[n, :D]  = sum_s E^T[s, n] y[s, :]      (unnormalized combine)
  outp[n, D]   = sum_s E^T[s, n]              (ones column appended to y)

Host-side unshard: the combine softmax normalizer is global over all
E*S slots, so out = (sum_c num_c + sum_c gdl_c * b2[e(c)]) / sum_c gdl_c
where num_c = outp_c[..., :D] and gdl_c = outp_c[..., D]. This also
folds in b2 exactly (per-expert combine mass times b2[e]).

All matmul operands are bf16 (PSUM accumulation stays f32): LDWEIGHTS
runs at full rate, 128-free matmuls run at 1 cyc/row (fp32r needs
free >= 256), DMA traffic and SBUF footprint halve. phi/w1/w2 are
SBUF-resident across batches. Emission is software-pipelined
(transpose(v+1) before slots(v), mlp1(h+1) before mlp2(h)) to keep the
PE matmul pipe gap-free (gaps de-ramp the PE clock 2.4 -> 1.2 GHz).
"""

import numpy as np
import ml_dtypes
from contextlib import ExitStack

import concourse.bass as bass
import concourse.tile as tile
from concourse import mybir
from concourse.bass import ts
from concourse.bass_utils import run_bass_kernel_spmd

F32 = mybir.dt.float32
BF16 = mybir.dt.bfloat16
AF = mybir.ActivationFunctionType

N_CORES = 8


# --------------------------------------------------------------------------
# Post-pass: the walrus build in this container enforces the ISA cap of one
# sync-wait per instruction (two for EventSemaphore); Tile's final drain can
# carry more. Hoist excess waits onto fresh same-engine NOPs.
# --------------------------------------------------------------------------
def _split_excess_waits(nc):
    caps = {"InstEventSemaphore": 2}
    n_new = 0
    for f in nc.m.functions:
        for bb in f.blocks:
            i = 0
            insts = bb.instructions
            while i < len(insts):
                ins = insts[i]
                si = ins.sync_info
                cap = caps.get(type(ins).__name__, 1)
                if si is not None and len(si.on_wait) > cap:
                    waits = list(si.on_wait)
                    keep, hoist = waits[-cap:], waits[:-cap]
                    new_nops = []
                    for w in hoist:
                        nop = mybir.InstNoOp(
                            name=nc.get_next_instruction_name(),
                            engine=ins.engine,
                            ins=[],
                            outs=[],
                            sync_info=mybir.SyncInfo(on_wait=[w], on_update=[]),
                        )
                        nc.register_instruction(nop)
                        new_nops.append(nop)
                    ins.sync_info = mybir.SyncInfo(
                        on_wait=keep, on_update=list(si.on_update)
                    )
                    insts[i:i] = new_nops
                    i += len(new_nops)
                    n_new += len(new_nops)
                i += 1
    return n_new


def _bank_splits(off, width, bank=512):
    """Split [off, off+width) at PSUM-bank (512 f32) boundaries."""
    out, cur = [], off
    while cur < off + width:
        nxt = min((cur // bank + 1) * bank, off + width)
        out.append((cur, nxt - cur))
        cur = nxt
    return out


def _emit_moe_kernel(nc, B, N, D, SL, H, act_fn=AF.Gelu_apprx_tanh):
    assert N % 512 == 0 and D % 128 == 0 and SL % 128 == 0 and H % 128 == 0
    Dc, SLc, Hc = D // 128, SL // 128, H // 128
    NT, NV = N // 512, N // 128
    OD = D + 2  # ones column (combine denom) + pad for alignment

    xT = nc.dram_tensor("xT", [B, Dc, 128, N], BF16, kind="ExternalInput").ap()
    xN = nc.dram_tensor("xN", [B, N, D], BF16, kind="ExternalInput").ap()
    phiT = nc.dram_tensor("phiT", [Dc, 128, SL], BF16, kind="ExternalInput").ap()
    w1 = nc.dram_tensor("w1", [D, H], BF16, kind="ExternalInput").ap()
    w2 = nc.dram_tensor("w2", [H, D], BF16, kind="ExternalInput").ap()
    b1 = nc.dram_tensor("b1", [Hc, 128], F32, kind="ExternalInput").ap()
    identI = nc.dram_tensor("identI", [128, 128], F32, kind="ExternalInput").ap()
    identIb = nc.dram_tensor("identIb", [128, 128], BF16, kind="ExternalInput").ap()
    outp = nc.dram_tensor("outp", [B, N, OD], BF16, kind="ExternalOutput").ap()

    with tile.TileContext(nc) as tc, ExitStack() as ctx:
        pool = lambda name, bufs, space="SBUF": ctx.enter_context(
            tc.tile_pool(name=name, bufs=bufs, space=space)
        )
        singles = pool("singles", 1)
        eT_pool = pool("eT", 1)
        xt_pool = pool("xT", 2)
        xn_pool = pool("xN", 8)
        Dt_pool = pool("Dt", 8)
        slots_pool = pool("slots", 1)
        h_pool = pool("h", 3)
        y_pool = pool("y", 1)
        dd_pool = pool("dd", 2)
        norm_pool = pool("norm", 1)
        out_pool = pool("out", 6)

        # PSUM pools are opened per phase (stack allocator + overlap-deps
        # on release): each phase gets the accumulator banks it needs and
        # the combine phase reuses the freed banks for a deep pso rotation.
        ACC = Dc * 512
        assert ACC == SLc * 768

        # ---- SBUF residents (loaded once, reused across all batches).
        # w1/w2 DMAs (9.4MB) are deferred into batch 0's first tile so they
        # don't queue ahead of the x tiles the first matmuls need.
        phiT_s = singles.tile([128, Dc, SL], BF16)
        nc.sync.dma_start(phiT_s[:], phiT.rearrange("k p m -> p k m"))
        w1_s = singles.tile([128, Dc, H], BF16)
        w2_s = singles.tile([128, Hc, D], BF16)
        b1_s = singles.tile([128, Hc], F32)
        nc.sync.dma_start(b1_s[:], b1.rearrange("o p -> p o"))
        ident = singles.tile([128, 128], F32)
        nc.sync.dma_start(ident[:], identI)
        ident_b = singles.tile([128, 128], BF16)
        nc.sync.dma_start(ident_b[:], identIb)
        # Broadcast-matmul weights: the PE rounds the K=1 contraction up to
        # 32 partitions and reads them all, so rows 1-31 must be REAL zeros
        # (garbage there would be accumulated into every output row).
        ones1 = singles.tile([32, 128], BF16)
        nc.vector.memset(ones1[:], 0.0)
        nc.vector.memset(ones1[0:1, :], 1.0)
        zbias = singles.tile([128, 1], F32)
        nc.vector.memset(zbias[:], 0.0)

        for b in range(B):
            # ---- phase 1+2 fused: logits/exp, per-block PE transposes, and
            # ---- unnormalized slots accumulation in one dense PE stream ----
            eT = eT_pool.tile([128, SLc, N], BF16)
            ddp = dd_pool.tile([128, SLc, NT], F32)
            ps12 = ctx12 = tc.tile_pool(name="ps12", bufs=2, space="PSUM")
            ps12 = ctx12.__enter__()
            psacc_ctx = tc.tile_pool(name="psacc", bufs=1, space="PSUM")
            ps_acc = psacc_ctx.__enter__()
            ps_small = ps12
            accS = ps_acc.tile([128, ACC], F32, tag="acc", name="accS")
            dts, xns = {}, {}

            def emit_T(v):
                psDt = ps_small.tile([128, 512], F32, tag="pss", name="psD")
                for s in range(SLc):
                    nc.tensor.matmul(
                        psDt[:, ts(s, 128)],
                        eT[:, s, ts(v, 128)],
                        ident_b[:],
                        start=True,
                        stop=True,
                    )
                Dt = Dt_pool.tile([128, SL], BF16)
                nc.vector.tensor_copy(Dt[:], psDt[:, :SL])
                dts[v] = Dt

            def emit_S(v):
                Dt, xn = dts.pop(v), xns.pop(v)
                for d in range(Dc):
                    nc.tensor.matmul(
                        accS[:, d * 512 : d * 512 + SL],
                        xn[:, ts(d, 128)],
                        Dt[:],
                        start=(v == 0),
                        stop=(v == NV - 1),
                    )

            # 1/ddenom broadcast-row chain, interleaved into the final tile's
            # T/S pairs so the PE never waits on the DVE/DMA latency:
            #   reduce+recip (DVE, after last exp) -> PE transpose
            #   [128,SLc]->[SLc,128] -> bf16 copy -> flatten DMAs ->
            #   1-partition ones matmul -> broadcast row in SBUF
            rdd = dd_pool.tile([128, SLc], F32, tag="rdd", name="rdd")
            pstS = norm_pool.tile([SLc, 128], BF16, tag="pstS", name="pstS")
            rddF = norm_pool.tile([1, SL], BF16, tag="rddF", name="rddF")
            rddB = norm_pool.tile([128, SL], F32, tag="rddB", name="rddB")

            def norm_chain_a():
                pst = ps_small.tile([128, 512], F32, tag="pss", name="psT")
                nc.tensor.transpose(pst[0:SLc, 0:128], rdd[:], ident[:])
                nc.vector.tensor_copy(pstS[:], pst[0:SLc, 0:128])
                # one DMA per source partition: a single rearranged
                # "p k -> (p k)" DMA under-reports its partition extent to
                # the Tile dependency tracker and races the copy above
                for k in range(SLc):
                    nc.sync.dma_start(rddF[0:1, ts(k, 128)], pstS[k : k + 1, :])

            def norm_chain_b():
                psB = ps_small.tile([128, 512], F32, tag="pss", name="psB")
                nc.tensor.matmul(
                    psB[:, :SL], ones1[0:1, :], rddF[:], start=True, stop=True
                )
                nc.vector.tensor_copy(rddB[:], psB[:, :SL])

            s_queue = []

            def emit_TS_for_tile(t, hooks=()):
                for i, v in enumerate(range(4 * t, 4 * t + 4)):
                    emit_T(v)
                    if s_queue:
                        emit_S(s_queue.pop(0))
                    s_queue.append(v)
                    for hook_i, hook in hooks:
                        if hook_i == i:
                            hook()

            for t in range(NT):
                xt = xt_pool.tile([128, Dc, 512], BF16)
                nc.sync.dma_start(
                    xt[:], xT[b, :, :, ts(t, 512)].rearrange("k p n -> p k n")
                )
                for s in range(SLc):
                    ps = ps_small.tile([128, 512], F32, tag="pss", name="psL")
                    for d in range(Dc):
                        nc.tensor.matmul(
                            ps[:],
                            phiT_s[:, d, ts(s, 128)],
                            xt[:, d, :],
                            start=(d == 0),
                            stop=(d == Dc - 1),
                        )
                    nc.scalar.activation(
                        eT[:, s, ts(t, 512)],
                        ps[:],
                        AF.Exp,
                        bias=zbias[:],
                        accum_out=ddp[:, s, t : t + 1],
                    )
                for v in range(4 * t, 4 * t + 4):
                    xn = xn_pool.tile([128, D], BF16)
                    nc.sync.dma_start(xn[:], xN[b, ts(v, 128), :])
                    xns[v] = xn
                if b == 0:
                    # stream the 9.4MB of resident weights in per-tile chunks
                    # so they never queue ahead of the x tiles phase 1 needs
                    hchunk = H // NT
                    nc.sync.dma_start(
                        w1_s[:, :, ts(t, hchunk)],
                        w1[:, ts(t, hchunk)].rearrange("(k p) m -> p k m", p=128),
                    )
                    kchunk = Hc // NT
                    nc.sync.dma_start(
                        w2_s[:, ts(t, kchunk), :],
                        w2[ts(t, kchunk * 128), :].rearrange(
                            "(k p) m -> p k m", p=128
                        ),
                    )
                if t == NT - 1:
                    nc.vector.reduce_sum(
                        rdd[:], ddp[:], axis=mybir.AxisListType.X
                    )
                    nc.vector.reciprocal(rdd[:], rdd[:])
                if t >= 1:
                    emit_TS_for_tile(t - 1)
            emit_TS_for_tile(NT - 1, hooks=((0, norm_chain_a), (3, norm_chain_b)))
            while s_queue:
                emit_S(s_queue.pop(0))

            # slots^T = slotsU^T * (1/ddenom), fused into the PSUM evacuation
            slotsT = slots_pool.tile([128, Dc, SL], BF16)
            for d in range(Dc):
                nc.vector.tensor_mul(
                    slotsT[:, d, :], accS[:, d * 512 : d * 512 + SL], rddB[:]
                )
            psacc_ctx.__exit__(None, None, None)
            ctx12.__exit__(None, None, None)
            ps3_ctx = tc.tile_pool(name="ps3", bufs=2, space="PSUM")
            ps_small = ps3_ctx.__enter__()
            psaccy_ctx = tc.tile_pool(name="psaccy", bufs=1, space="PSUM")
            ps_acc = psaccy_ctx.__enter__()

            # ---- phase 3: expert MLP; y accumulated with s on partitions ----
            accY = ps_acc.tile([128, ACC], F32, tag="acc", name="accY")

            def emit_h1(h):
                psh = ps_small.tile([128, 512], F32, tag="pss", name="psH")
                for d in range(Dc):
                    nc.tensor.matmul(
                        psh[:, :SL],
                        w1_s[:, d, ts(h, 128)],
                        slotsT[:, d, :],
                        start=(d == 0),
                        stop=(d == Dc - 1),
                    )
                ht = h_pool.tile([128, SL], BF16)
                nc.scalar.activation(
                    ht[:], psh[:, :SL], act_fn, bias=b1_s[:, h : h + 1]
                )
                return ht

            def emit_h2(h, ht):
                for s in range(SLc):
                    for off, sz in _bank_splits(s * D, D):
                        # start=True clears the has_written bits of the WHOLE
                        # 2KB PSUM bank: only the bank-leading piece may carry
                        # it. A same-bank follower piece relies on that clear:
                        # its first start=False matmul overwrites (bit clear),
                        # later ones accumulate (bit set).
                        bank_first = off % 512 == 0
                        nc.tensor.matmul(
                            accY[:, off : off + sz],
                            ht[:, ts(s, 128)],
                            w2_s[:, h, off - s * D : off - s * D + sz],
                            start=(h == 0 and bank_first),
                            stop=(h == Hc - 1),
                            skip_group_check=not bank_first,
                        )

            pend_h = None
            for h in range(Hc):
                cur = emit_h1(h)
                if pend_h is not None:
                    emit_h2(h - 1, pend_h)
                pend_h = cur
            emit_h2(Hc - 1, pend_h)

            # ---- y_aug assembly: y columns + ones column for gdl ----
            y_aug = y_pool.tile([128, SLc, OD], BF16)
            nc.vector.memset(y_aug[:, :, D : D + 1], 1.0)
            nc.vector.memset(y_aug[:, :, D + 1 : D + 2], 0.0)
            for s in range(SLc):
                srcY = accY[:, s * D : (s + 1) * D]
                h1 = D // 2
                nc.vector.tensor_copy(y_aug[:, s, :h1], srcY[:, :h1])
                nc.scalar.copy(y_aug[:, s, h1:D], srcY[:, h1:])

            psaccy_ctx.__exit__(None, None, None)
            ps3_ctx.__exit__(None, None, None)
            ps4_ctx = tc.tile_pool(name="ps4", bufs=6, space="PSUM")
            ps_small = ps4_ctx.__enter__()
            # ---- phase 4: combine partials + local denominator ----
            for v in range(NV):
                ot = out_pool.tile([128, OD], BF16)
                for gi, (off, sz) in enumerate(((0, 512), (512, OD - 512))):
                    pso = ps_small.tile([128, 512], F32, tag="pss", name="psO")
                    for s in range(SLc):
                        nc.tensor.matmul(
                            pso[:, :sz],
                            eT[:, s, ts(v, 128)],
                            y_aug[:, s, off : off + sz],
                            start=(s == 0),
                            stop=(s == SLc - 1),
                        )
                    # alternate evacuation engines so neither serializes
                    # the 2-buf PSUM pool rotation
                    if gi == 0:
                        nc.scalar.copy(ot[:, off : off + sz], pso[:, :sz])
                    else:
                        nc.vector.tensor_copy(ot[:, off : off + sz], pso[:, :sz])
                nc.sync.dma_start(outp[b, ts(v, 128), :], ot[:])
            ps4_ctx.__exit__(None, None, None)

    return nc


def _make_core_inputs(x, phi, w1, b1, w2, n_cores=N_CORES):
    B, N, Dd = x.shape
    E, S, _ = phi.shape
    H = w1.shape[2]
    halves = n_cores // E
    SL = S // halves
    Dc, Hc = Dd // 128, H // 128
    bf = ml_dtypes.bfloat16
    ident_f32 = np.eye(128, dtype=np.float32)
    ident_bf16 = np.eye(128, dtype=bf)
    xT_full = np.ascontiguousarray(
        x.transpose(0, 2, 1).astype(bf)
    ).reshape(B, Dc, 128, N)
    x_c = np.ascontiguousarray(x.astype(bf))
    in_maps = []
    for c in range(n_cores):
        e, hh = c // halves, c % halves
        phi_loc = phi[e, hh * SL : (hh + 1) * SL, :]
        phiT = np.ascontiguousarray(phi_loc.T.astype(bf)).reshape(Dc, 128, SL)
        in_maps.append(
            {
                "xT": xT_full,
                "xN": x_c,
                "phiT": phiT,
                "w1": np.ascontiguousarray(w1[e].astype(bf)),
                "w2": np.ascontiguousarray(w2[e].astype(bf)),
                "b1": np.ascontiguousarray(b1[e]).reshape(Hc, 128),
                "identI": ident_f32,
                "identIb": ident_bf16,
            }
        )
    return in_maps


def _combine_core_outputs(outs, b2, n_cores=N_CORES):
    E, D = b2.shape
    halves = n_cores // E
    num = np.zeros(outs[0]["outp"][..., :D].shape, dtype=np.float64)
    den = np.zeros(outs[0]["outp"][..., D].shape, dtype=np.float64)
    for c, r in enumerate(outs):
        e = c // halves
        gdl = r["outp"][..., D].astype(np.float64)
        num += r["outp"][..., :D]
        num += gdl[..., None] * b2[e].astype(np.float64)[None, None, :]
        den += gdl
    return (num / den[..., None]).astype(np.float32)


def kernel(x, phi, w1, b1, w2, b2):
    x = np.asarray(x, dtype=np.float32)
    phi = np.asarray(phi, dtype=np.float32)
    w1 = np.asarray(w1, dtype=np.float32)
    b1 = np.asarray(b1, dtype=np.float32)
    w2 = np.asarray(w2, dtype=np.float32)
    b2 = np.asarray(b2, dtype=np.float32)

    B, N, D = x.shape
    E, S, _ = phi.shape
    H = w1.shape[2]
    SL = S // (N_CORES // E)

    nc = bass.Bass(
        "TRN2", target_bir_lowering=False, debug=False, num_devices=N_CORES
    )
    _emit_moe_kernel(nc, B, N, D, SL, H)
    _split_excess_waits(nc)

    in_maps = _make_core_inputs(x, phi, w1, b1, w2)
    res = run_bass_kernel_spmd(nc, in_maps, core_ids=list(range(N_CORES)))
    return _combine_core_outputs(res.results, b2)
